# revision 7
# baseline (speedup 1.0000x reference)
"""Trainium2 Bass kernel for nn_MultiHeadAttentionLayer (GNN message passing).

Contract: kernel(**inputs) takes the FULL unsharded inputs (as produced by
setup_inputs()) and returns the FULL output [N, H, D] float32.

Strategy (8 NeuronCores, SPMD, no collectives, no gather):
  - dst == arange(E) % N (spec fill "arange"), so node n receives exactly
    E/N == 8 edges: e = k*N + n, k = 0..7.  Destination nodes are sharded
    across cores (N/8 nodes each); the host lays out the source-node
    features IN EDGE ORDER (pure indexing, no FLOPs), so the device streams
    them sequentially - the scatter/gather of the GNN disappears entirely.
  - Per (m-tile of 128 nodes, k): project the 128 gathered edge features
    through [Wk|Wv] (bf16, two 128-contraction matmuls) into PSUM.
    Scores: prod+reduce on DVE, exp on ACT.  Scores are pre-divided by
    z = sum_k score, so the aggregation needs no output divide.
  - V for the jaccard heads is scaled during PSUM eviction on the Scalar
    engine (activation Copy with per-partition scale = jaccard/z_jac);
    V for the QK heads is scaled on DVE.  The 8-edge accumulation is
    identity-matmuls into PSUM.  Blocks of 7 m-tiles are software-pipelined
    one deep so PE stays dense.
"""

import sys

import numpy as np

for _p in ("/opt/trn_rl_repo",):
    if _p not in sys.path:
        sys.path.insert(0, _p)

# --- problem constants (hardcoded per spec; kernel.py must be self-contained)
N_NODES = 50000
N_EDGES = 400000
IN_DIM = 256
OUT_DIM = 32
NUM_HEADS = 8
H2 = NUM_HEADS // 2
N_CORES = 8
P = 128

CLIP = 5.0 * np.sqrt(np.float32(32.0))  # clip on the raw dot product
SCALE = float(1.0 / np.sqrt(np.float32(32.0)))


class Cfg:
    def __init__(self, n_nodes=N_NODES, n_edges=N_EDGES, n_cores=N_CORES):
        assert n_edges == 8 * n_nodes
        self.N = n_nodes
        self.E = n_edges
        self.NC = n_cores
        assert n_nodes % n_cores == 0
        self.NPC = n_nodes // n_cores          # nodes per core
        self.M = -(-self.NPC // P)             # 128-node tiles per core
        self.NPAD = self.M * P                 # padded nodes per core
        self.BM = 7 if self.M % 7 == 0 else 1  # m-tiles per block
        self.NB = self.M // self.BM            # blocks


FULL_CFG = Cfg()


# --------------------------------------------------------------------------
# device program
# --------------------------------------------------------------------------

def build_program(cfg: Cfg, repeat: int = 1):
    import concourse.bacc as bacc
    import concourse.mybir as mybir
    import concourse.tile as tile
    from concourse.masks import make_identity

    f32 = mybir.dt.float32
    bf16 = mybir.dt.bfloat16
    M, BM, NB = cfg.M, cfg.BM, cfg.NB

    nc = bacc.Bacc(
        "TRN2",
        target_bir_lowering=False,
        debug=False,
        enable_asserts=False,
        num_devices=cfg.NC,
    )

    hT_own = nc.dram_tensor("hT_own", [2, P, cfg.NPAD], bf16, kind="ExternalInput")
    he_t = nc.dram_tensor("he_t", [8, 2, P, cfg.NPAD], bf16, kind="ExternalInput")
    w_kv = nc.dram_tensor("w_kv", [P, 2, 384], bf16, kind="ExternalInput")
    w_q = nc.dram_tensor("w_q", [P, 2, P], bf16, kind="ExternalInput")
    jac_t = nc.dram_tensor("jac", [P, 8, M], f32, kind="ExternalInput")
    out_t = nc.dram_tensor("out", [P, M * 256], f32, kind="ExternalOutput")

    mult = mybir.AluOpType.mult
    add = mybir.AluOpType.add
    amin = mybir.AluOpType.min
    amax = mybir.AluOpType.max
    EC = BM * P  # edge columns per (block, k)

    with tile.TileContext(nc) as tc:
        with tc.tile_pool(name="const", bufs=1) as const:
          for _rep in range(repeat):
            wkv_sb = const.tile([P, 2, 384], bf16, tag="wkv")
            wq_sb = const.tile([P, 2, P], bf16, tag="wq")
            jac_sb = const.tile([P, 8, M], f32, tag="jacsb")
            z47r = const.tile([P, M], f32, tag="z47r")
            jacw = const.tile([P, 8, M], f32, tag="jacw")
            qloc = const.tile([P, M, P], bf16, tag="qloc")
            ident = const.tile([P, P], bf16, tag="ident")

            nc.sync.dma_start(out=wkv_sb[:], in_=w_kv[:])
            nc.sync.dma_start(out=wq_sb[:], in_=w_q[:])
            nc.sync.dma_start(out=jac_sb[:], in_=jac_t[:])
            make_identity(nc, ident[:])

            # jacw = jac / sum_k jac  (pre-divided jaccard-head scores)
            nc.vector.tensor_reduce(
                out=z47r[:], in_=jac_sb[:].rearrange("p k m -> p m k"),
                axis=mybir.AxisListType.X, op=add)
            nc.vector.reciprocal(z47r[:], z47r[:])
            nc.vector.tensor_tensor(
                out=jacw[:], in0=jac_sb[:],
                in1=z47r[:].unsqueeze(1).to_broadcast([P, 8, M]), op=mult)

            # ---- phase A: Q projection of own node shard ------------------
            with (
                tc.tile_pool(name="pa", bufs=1) as pa,
                tc.tile_pool(name="pa_ps", bufs=2, space="PSUM") as pap,
            ):
                hto = pa.tile([P, 2, cfg.NPAD], bf16)
                nc.sync.dma_start(out=hto[:], in_=hT_own[:].rearrange("h f e -> f h e"))
                for t in range(M):
                    ps_q = pap.tile([P, P], f32)
                    nc.tensor.matmul(
                        out=ps_q[:], lhsT=hto[:, 0, t * P:(t + 1) * P],
                        rhs=wq_sb[:, 0, :], start=True, stop=False)
                    nc.tensor.matmul(
                        out=ps_q[:], lhsT=hto[:, 1, t * P:(t + 1) * P],
                        rhs=wq_sb[:, 1, :], start=False, stop=True)
                    nc.vector.tensor_copy(out=qloc[:, t, :], in_=ps_q[:])

            # ---- phase B: edge projection + attention, pipelined ----------
            with (
                tc.tile_pool(name="he", bufs=3) as hep,
                tc.tile_pool(name="kv_ps", bufs=2, space="PSUM") as kvp,
                tc.tile_pool(name="wv_ps", bufs=1, space="PSUM") as wvp,
                tc.tile_pool(name="pb", bufs=2) as pb,
                tc.tile_pool(name="po", bufs=2) as po,
            ):
                state = {}

                def emit_front(b):
                    """DMA + projection + prod + evictions for block b."""
                    gm0 = b * BM
                    prodb = pb.tile([P, BM, 8, P], bf16, tag="prodb")
                    vkq = pb.tile([P, BM, 8, P], bf16, tag="vkq")
                    sclb = pb.tile([P, 8, BM, 256], bf16, tag="sclb")
                    dots = pb.tile([P, BM, 8, 4], f32, tag="dots")
                    for k in range(8):
                        he = hep.tile([P, 2, EC], bf16, tag="he")
                        nc.sync.dma_start(
                            out=he[:],
                            in_=he_t[k, :, :, gm0 * P:gm0 * P + EC]
                                .rearrange("h f e -> f h e"))
                        for mp in range((BM + 1) // 2):
                            w = min(2, BM - 2 * mp)
                            kp = kvp.tile([P, 2, 512], f32, tag="kvps")
                            for i in range(w):
                                m = 2 * mp + i
                                nc.tensor.matmul(
                                    out=kp[:, i, 0:384],
                                    lhsT=he[:, 0, m * P:(m + 1) * P],
                                    rhs=wkv_sb[:, 0, :], start=True, stop=False)
                                nc.tensor.matmul(
                                    out=kp[:, i, 0:384],
                                    lhsT=he[:, 1, m * P:(m + 1) * P],
                                    rhs=wkv_sb[:, 1, :], start=False, stop=True)
                            # scores: K * Q  (bf16 products, reduced later)
                            nc.vector.tensor_tensor(
                                out=prodb[:, 2 * mp:2 * mp + w, k, :],
                                in0=kp[:, 0:w, 0:P],
                                in1=qloc[:, gm0 + 2 * mp:gm0 + 2 * mp + w, :],
                                op=mult)
                            # evict V (QK heads) to bf16
                            nc.scalar.copy(
                                out=vkq[:, 2 * mp:2 * mp + w, k, :],
                                in_=kp[:, 0:w, P:256])
                            # evict V (jac heads), scaled by jac/z on the fly
                            for i in range(w):
                                m = 2 * mp + i
                                nc.scalar.activation(
                                    out=sclb[:, k, m, P:256],
                                    in_=kp[:, i, 256:384],
                                    func=mybir.ActivationFunctionType.Copy,
                                    scale=jacw[:, k, gm0 + m:gm0 + m + 1])
                    state[b] = (prodb, vkq, sclb, dots)

                def emit_back(b):
                    """scores -> sew -> scl -> accumulate -> store, block b."""
                    gm0 = b * BM
                    prodb, vkq, sclb, dots = state.pop(b)
                    for m in range(BM):
                        nc.vector.tensor_reduce(
                            out=dots[:, m, :, :],
                            in_=prodb[:, m, :, :].rearrange(
                                "p k (h c) -> p k h c", c=32),
                            axis=mybir.AxisListType.X, op=add)
                    se = pb.tile([P, BM, 8, 4], f32, tag="se")
                    zq = pb.tile([P, BM, 4], f32, tag="zq")
                    sew = pb.tile([P, BM, 8, 4], bf16, tag="sew")
                    nc.vector.tensor_scalar(
                        out=dots[:], in0=dots[:], scalar1=float(CLIP),
                        scalar2=float(-CLIP), op0=amin, op1=amax)
                    nc.scalar.activation(
                        out=se[:], in_=dots[:],
                        func=mybir.ActivationFunctionType.Exp, scale=SCALE)
                    nc.vector.tensor_reduce(
                        out=zq[:], in_=se[:].rearrange("p m k h -> p m h k"),
                        axis=mybir.AxisListType.X, op=add)
                    nc.vector.reciprocal(zq[:], zq[:])
                    nc.vector.tensor_tensor(
                        out=sew[:], in0=se[:],
                        in1=zq[:].unsqueeze(2).to_broadcast([P, BM, 8, 4]),
                        op=mult)
                    for m in range(BM):
                        nc.vector.tensor_tensor(
                            out=sclb[:, :, m, 0:P].rearrange(
                                "p k (h c) -> p k h c", c=32),
                            in0=vkq[:, m, :, :].rearrange(
                                "p k (h c) -> p k h c", c=32),
                            in1=sew[:, m, :, :].unsqueeze(3)
                                .to_broadcast([P, 8, 4, 32]),
                            op=mult)
                    wv = wvp.tile([P, BM, 256], f32, tag="wv")
                    for k in range(8):
                        for mp in range((BM + 1) // 2):
                            w = min(2, BM - 2 * mp)
                            nc.tensor.matmul(
                                out=wv[:, 2 * mp:2 * mp + w, :].rearrange(
                                    "p m f -> p (m f)"),
                                lhsT=ident[:],
                                rhs=sclb[:, k, 2 * mp:2 * mp + w, :].rearrange(
                                    "p m f -> p (m f)"),
                                start=(k == 0), stop=(k == 7))
                    ost = po.tile([P, BM, 256], f32, tag="ost")
                    nc.scalar.copy(out=ost[:], in_=wv[:])
                    nc.sync.dma_start(
                        out=out_t[:, gm0 * 256:(gm0 + BM) * 256],
                        in_=ost[:].rearrange("p m f -> p (m f)"))

                for b in range(NB + 1):
                    if b < NB:
                        emit_front(b)
                    if b >= 1:
                        emit_back(b - 1)

    nc.compile()
    return nc


# --------------------------------------------------------------------------
# host-side sharding / assembly (layout only, no FLOPs)
# --------------------------------------------------------------------------

def host_prepare(cfg: Cfg, h, src, jaccard, Wq, Wk, Wv):
    import ml_dtypes

    f32 = np.float32
    bf16 = ml_dtypes.bfloat16
    hT = np.ascontiguousarray(h.T.astype(f32, copy=False)).astype(bf16)

    def chunk2(w_t, width):  # [256, width] -> [128, 2, width]
        return np.ascontiguousarray(
            w_t.reshape(2, P, width).transpose(1, 0, 2)).astype(bf16)

    w_kv = chunk2(np.concatenate([Wk.T, Wv.T], axis=1).astype(f32), 384)
    w_q = chunk2(np.ascontiguousarray(Wq.T).astype(f32), P)

    # slot s = m*128 + p  <->  node n = c*NPC + s  (s < NPC valid)
    slots = np.arange(cfg.NPAD)
    s_grid = slots.reshape(cfg.M, P).T                     # [128, M]
    valid = s_grid < cfg.NPC

    in_maps = []
    for c in range(cfg.NC):
        base = c * cfg.NPC
        nodes = base + np.minimum(slots, cfg.NPC - 1)      # [NPAD]
        he = np.empty((8, 2, P, cfg.NPAD), dtype=bf16)
        jac = np.ones((P, 8, cfg.M), dtype=f32)
        for k in range(8):
            e = k * cfg.N + nodes
            he[k] = hT[:, src[e]].reshape(2, P, cfg.NPAD)
            jac[:, k, :] = np.where(valid, jaccard[e][s_grid], 1.0)
        hT_own = np.zeros((2, P, cfg.NPAD), dtype=bf16)
        span = min(cfg.NPAD, cfg.N - base)
        hT_own[:, :, :span] = hT[:, base:base + span].reshape(2, P, span)
        in_maps.append({
            "hT_own": hT_own,
            "he_t": he,
            "w_kv": w_kv,
            "w_q": w_q,
            "jac": np.ascontiguousarray(jac),
        })
    return in_maps


def assemble_output(cfg: Cfg, results):
    out = np.empty((cfg.N, NUM_HEADS, OUT_DIM), dtype=np.float32)
    for c, r in enumerate(results):
        shard = r["out"].reshape(P, cfg.M, 256).transpose(1, 0, 2).reshape(
            cfg.NPAD, 256)[:cfg.NPC]
        out[c * cfg.NPC:(c + 1) * cfg.NPC] = shard.reshape(
            cfg.NPC, NUM_HEADS, OUT_DIM)
    return out


# --------------------------------------------------------------------------
# numpy fallback (used only if inputs don't match the spec'd structure)
# --------------------------------------------------------------------------

def _numpy_reference(h, src, dst, jaccard, Wq, bq, Wk, bk, Wv, bv):
    N = h.shape[0]
    E = src.shape[0]
    h = h.astype(np.float32)
    Qh = (h @ Wq.T + bq).reshape(N, H2, OUT_DIM)
    Kh = (h @ Wk.T + bk).reshape(N, H2, OUT_DIM)
    Vh = (h @ Wv.T + bv).reshape(N, NUM_HEADS, OUT_DIM)
    score = np.sum(Kh[src] * Qh[dst], axis=-1, keepdims=True)
    score = np.exp(np.clip(score / np.sqrt(np.float32(OUT_DIM)), -5.0, 5.0))
    jac = np.broadcast_to(jaccard[:, None, None], (E, H2, 1))
    score_new = np.concatenate([score, jac], axis=1).astype(np.float32)
    contrib = (Vh[src] * score_new).astype(np.float32)
    wV = np.zeros((N, NUM_HEADS, OUT_DIM), dtype=np.float32)
    z = np.zeros((N, NUM_HEADS, 1), dtype=np.float32)
    np.add.at(wV, dst, contrib)
    np.add.at(z, dst, score_new)
    return wV / z


# --------------------------------------------------------------------------
# entry point
# --------------------------------------------------------------------------

_PROGRAM_CACHE = {}


def _get_program(cfg: Cfg):
    key = (cfg.N, cfg.E, cfg.NC)
    if key not in _PROGRAM_CACHE:
        _PROGRAM_CACHE[key] = build_program(cfg)
    return _PROGRAM_CACHE[key]


def _structure_ok(h, src, dst, jaccard, Wq, bq, Wk, bk, Wv, bv):
    if h.shape != (N_NODES, IN_DIM) or src.shape != (N_EDGES,):
        return False
    if Wq.shape != (H2 * OUT_DIM, IN_DIM) or Wv.shape != (NUM_HEADS * OUT_DIM, IN_DIM):
        return False
    if np.any(bq) or np.any(bk) or np.any(bv):
        return False
    if not np.array_equal(
            np.asarray(dst, dtype=np.int64),
            np.arange(N_EDGES, dtype=np.int64) % N_NODES):
        return False
    if src.min() < 0 or src.max() >= N_NODES:
        return False
    return True


def run_on_hw(inputs):
    from concourse.bass2jax import run_bass_via_pjrt

    cfg = FULL_CFG
    nc = _get_program(cfg)
    in_maps = host_prepare(
        cfg, inputs["h"], inputs["src"], inputs["jaccard"],
        inputs["Wq"], inputs["Wk"], inputs["Wv"])
    results = run_bass_via_pjrt(nc, in_maps, n_cores=cfg.NC)
    return assemble_output(cfg, results), results


def kernel(**inputs) -> np.ndarray:
    args = {k: np.asarray(v) for k, v in inputs.items()}
    if not _structure_ok(**args):
        return _numpy_reference(**args)
    out, _ = run_on_hw(args)
    return out


if __name__ == "__main__":
    print("building full program...")
    nc = _get_program(FULL_CFG)
    print("ok")


# revision 8
# speedup vs baseline: 1.0089x; 1.0089x over previous
"""Trainium2 Bass kernel for nn_MultiHeadAttentionLayer (GNN message passing).

Contract: kernel(**inputs) takes the FULL unsharded inputs (as produced by
setup_inputs()) and returns the FULL output [N, H, D] float32.

Strategy (8 NeuronCores, SPMD, no collectives, no gather):
  - dst == arange(E) % N (spec fill "arange"), so node n receives exactly
    E/N == 8 edges: e = k*N + n, k = 0..7.  Destination nodes are sharded
    across cores (N/8 nodes each); the host lays out the source-node
    features IN EDGE ORDER (pure indexing, no FLOPs), so the device streams
    them sequentially - the scatter/gather of the GNN disappears entirely.
  - Per (m-tile of 128 nodes, k): project the 128 gathered edge features
    through [Wk|Wv] (bf16, two 128-contraction matmuls) into PSUM.
    Scores: prod+reduce on DVE, exp on ACT.  Scores are pre-divided by
    z = sum_k score, so the aggregation needs no output divide.
  - V for the jaccard heads is scaled during PSUM eviction on the Scalar
    engine (activation Copy with per-partition scale = jaccard/z_jac);
    V for the QK heads is scaled on DVE.  The 8-edge accumulation is
    identity-matmuls into PSUM.  Blocks of 7 m-tiles are software-pipelined
    one deep so PE stays dense.
"""

import sys

import numpy as np

for _p in ("/opt/trn_rl_repo",):
    if _p not in sys.path:
        sys.path.insert(0, _p)

# --- problem constants (hardcoded per spec; kernel.py must be self-contained)
N_NODES = 50000
N_EDGES = 400000
IN_DIM = 256
OUT_DIM = 32
NUM_HEADS = 8
H2 = NUM_HEADS // 2
N_CORES = 8
P = 128

CLIP = 5.0 * np.sqrt(np.float32(32.0))  # clip on the raw dot product
SCALE = float(1.0 / np.sqrt(np.float32(32.0)))


class Cfg:
    def __init__(self, n_nodes=N_NODES, n_edges=N_EDGES, n_cores=N_CORES):
        assert n_edges == 8 * n_nodes
        self.N = n_nodes
        self.E = n_edges
        self.NC = n_cores
        assert n_nodes % n_cores == 0
        self.NPC = n_nodes // n_cores          # nodes per core
        self.M = -(-self.NPC // P)             # 128-node tiles per core
        self.NPAD = self.M * P                 # padded nodes per core
        self.BM = 7 if self.M % 7 == 0 else 1  # m-tiles per block
        self.NB = self.M // self.BM            # blocks


FULL_CFG = Cfg()


# --------------------------------------------------------------------------
# device program
# --------------------------------------------------------------------------

def build_program(cfg: Cfg, repeat: int = 1):
    import concourse.bacc as bacc
    import concourse.mybir as mybir
    import concourse.tile as tile
    from concourse.masks import make_identity

    f32 = mybir.dt.float32
    bf16 = mybir.dt.bfloat16
    M, BM, NB = cfg.M, cfg.BM, cfg.NB

    nc = bacc.Bacc(
        "TRN2",
        target_bir_lowering=False,
        debug=False,
        enable_asserts=False,
        num_devices=cfg.NC,
    )

    hT_own = nc.dram_tensor("hT_own", [2, P, cfg.NPAD], bf16, kind="ExternalInput")
    he_t = nc.dram_tensor("he_t", [8, 2, P, cfg.NPAD], bf16, kind="ExternalInput")
    w_kv = nc.dram_tensor("w_kv", [P, 2, 384], bf16, kind="ExternalInput")
    w_q = nc.dram_tensor("w_q", [P, 2, P], bf16, kind="ExternalInput")
    jac_t = nc.dram_tensor("jac", [P, 8, M], f32, kind="ExternalInput")
    out_t = nc.dram_tensor("out", [P, M * 256], f32, kind="ExternalOutput")

    mult = mybir.AluOpType.mult
    add = mybir.AluOpType.add
    amin = mybir.AluOpType.min
    amax = mybir.AluOpType.max
    EC = BM * P  # edge columns per (block, k)

    with tile.TileContext(nc) as tc:
        with tc.tile_pool(name="const", bufs=1) as const:
          for _rep in range(repeat):
            wkv_sb = const.tile([P, 2, 384], bf16, tag="wkv")
            wq_sb = const.tile([P, 2, P], bf16, tag="wq")
            jac_sb = const.tile([P, 8, M], f32, tag="jacsb")
            z47r = const.tile([P, M], f32, tag="z47r")
            jacw = const.tile([P, 8, M], f32, tag="jacw")
            qloc = const.tile([P, M, P], bf16, tag="qloc")
            ident = const.tile([P, P], bf16, tag="ident")

            nc.sync.dma_start(out=wkv_sb[:], in_=w_kv[:])
            nc.sync.dma_start(out=wq_sb[:], in_=w_q[:])
            nc.sync.dma_start(out=jac_sb[:], in_=jac_t[:])
            make_identity(nc, ident[:])

            # jacw = jac / sum_k jac  (pre-divided jaccard-head scores)
            nc.vector.tensor_reduce(
                out=z47r[:], in_=jac_sb[:].rearrange("p k m -> p m k"),
                axis=mybir.AxisListType.X, op=add)
            nc.vector.reciprocal(z47r[:], z47r[:])
            nc.vector.tensor_tensor(
                out=jacw[:], in0=jac_sb[:],
                in1=z47r[:].unsqueeze(1).to_broadcast([P, 8, M]), op=mult)

            # ---- phase A: Q projection of own node shard ------------------
            with (
                tc.tile_pool(name="pa", bufs=1) as pa,
                tc.tile_pool(name="pa_ps", bufs=2, space="PSUM") as pap,
            ):
                hto = pa.tile([P, 2, cfg.NPAD], bf16)
                nc.sync.dma_start(out=hto[:], in_=hT_own[:].rearrange("h f e -> f h e"))
                for t in range(M):
                    ps_q = pap.tile([P, P], f32)
                    nc.tensor.matmul(
                        out=ps_q[:], lhsT=hto[:, 0, t * P:(t + 1) * P],
                        rhs=wq_sb[:, 0, :], start=True, stop=False)
                    nc.tensor.matmul(
                        out=ps_q[:], lhsT=hto[:, 1, t * P:(t + 1) * P],
                        rhs=wq_sb[:, 1, :], start=False, stop=True)
                    nc.vector.tensor_copy(out=qloc[:, t, :], in_=ps_q[:])

            # ---- phase B: edge projection + attention, pipelined ----------
            with (
                tc.tile_pool(name="he", bufs=3) as hep,
                tc.tile_pool(name="kv_ps", bufs=2, space="PSUM") as kvp,
                tc.tile_pool(name="wv_ps", bufs=1, space="PSUM") as wvp,
                tc.tile_pool(name="pb", bufs=2) as pb,
                tc.tile_pool(name="po", bufs=2) as po,
            ):
                state = {}

                def emit_front(b):
                    """DMA + projection + prod + evictions for block b."""
                    gm0 = b * BM
                    prodb = pb.tile([P, BM, 8, P], bf16, tag="prodb")
                    vkq = pb.tile([P, BM, 8, P], bf16, tag="vkq")
                    sclb = pb.tile([P, 8, BM, 256], bf16, tag="sclb")
                    dots = pb.tile([P, BM, 8, 4], f32, tag="dots")
                    for k in range(8):
                        he = hep.tile([P, 2, EC], bf16, tag="he")
                        nc.sync.dma_start(
                            out=he[:],
                            in_=he_t[k, :, :, gm0 * P:gm0 * P + EC]
                                .rearrange("h f e -> f h e"))
                        for mp in range((BM + 1) // 2):
                            w = min(2, BM - 2 * mp)
                            kp = kvp.tile([P, 2, 512], f32, tag="kvps")
                            for i in range(w):
                                m = 2 * mp + i
                                nc.tensor.matmul(
                                    out=kp[:, i, 0:384],
                                    lhsT=he[:, 0, m * P:(m + 1) * P],
                                    rhs=wkv_sb[:, 0, :], start=True, stop=False)
                                nc.tensor.matmul(
                                    out=kp[:, i, 0:384],
                                    lhsT=he[:, 1, m * P:(m + 1) * P],
                                    rhs=wkv_sb[:, 1, :], start=False, stop=True)
                            # scores: K * Q  (bf16 products, reduced later)
                            nc.vector.tensor_tensor(
                                out=prodb[:, 2 * mp:2 * mp + w, k, :],
                                in0=kp[:, 0:w, 0:P],
                                in1=qloc[:, gm0 + 2 * mp:gm0 + 2 * mp + w, :],
                                op=mult)
                            # evict V (QK heads) to bf16; split across the
                            # Scalar and Vector engines to balance load
                            if k >= 5:
                                nc.vector.tensor_copy(
                                    out=vkq[:, 2 * mp:2 * mp + w, k, :],
                                    in_=kp[:, 0:w, P:256])
                            else:
                                nc.scalar.copy(
                                    out=vkq[:, 2 * mp:2 * mp + w, k, :],
                                    in_=kp[:, 0:w, P:256])
                            # evict V (jac heads), scaled by jac/z on the fly
                            for i in range(w):
                                m = 2 * mp + i
                                nc.scalar.activation(
                                    out=sclb[:, k, m, P:256],
                                    in_=kp[:, i, 256:384],
                                    func=mybir.ActivationFunctionType.Copy,
                                    scale=jacw[:, k, gm0 + m:gm0 + m + 1])
                    state[b] = (prodb, vkq, sclb, dots)

                def emit_back(b):
                    """scores -> sew -> scl -> accumulate -> store, block b."""
                    gm0 = b * BM
                    prodb, vkq, sclb, dots = state.pop(b)
                    for m in range(BM):
                        nc.vector.tensor_reduce(
                            out=dots[:, m, :, :],
                            in_=prodb[:, m, :, :].rearrange(
                                "p k (h c) -> p k h c", c=32),
                            axis=mybir.AxisListType.X, op=add)
                    se = pb.tile([P, BM, 8, 4], f32, tag="se")
                    zq = pb.tile([P, BM, 4], f32, tag="zq")
                    sew = pb.tile([P, BM, 8, 4], bf16, tag="sew")
                    nc.vector.tensor_scalar(
                        out=dots[:], in0=dots[:], scalar1=float(CLIP),
                        scalar2=float(-CLIP), op0=amin, op1=amax)
                    nc.scalar.activation(
                        out=se[:], in_=dots[:],
                        func=mybir.ActivationFunctionType.Exp, scale=SCALE)
                    nc.vector.tensor_reduce(
                        out=zq[:], in_=se[:].rearrange("p m k h -> p m h k"),
                        axis=mybir.AxisListType.X, op=add)
                    nc.vector.reciprocal(zq[:], zq[:])
                    nc.vector.tensor_tensor(
                        out=sew[:], in0=se[:],
                        in1=zq[:].unsqueeze(2).to_broadcast([P, BM, 8, 4]),
                        op=mult)
                    for m in range(BM):
                        nc.vector.tensor_tensor(
                            out=sclb[:, :, m, 0:P].rearrange(
                                "p k (h c) -> p k h c", c=32),
                            in0=vkq[:, m, :, :].rearrange(
                                "p k (h c) -> p k h c", c=32),
                            in1=sew[:, m, :, :].unsqueeze(3)
                                .to_broadcast([P, 8, 4, 32]),
                            op=mult)
                    wv = wvp.tile([P, BM, 256], f32, tag="wv")
                    for k in range(8):
                        for mp in range((BM + 1) // 2):
                            w = min(2, BM - 2 * mp)
                            nc.tensor.matmul(
                                out=wv[:, 2 * mp:2 * mp + w, :].rearrange(
                                    "p m f -> p (m f)"),
                                lhsT=ident[:],
                                rhs=sclb[:, k, 2 * mp:2 * mp + w, :].rearrange(
                                    "p m f -> p (m f)"),
                                start=(k == 0), stop=(k == 7))
                    ost = po.tile([P, BM, 256], f32, tag="ost")
                    nc.scalar.copy(out=ost[:], in_=wv[:])
                    nc.sync.dma_start(
                        out=out_t[:, gm0 * 256:(gm0 + BM) * 256],
                        in_=ost[:].rearrange("p m f -> p (m f)"))

                for b in range(NB + 1):
                    if b < NB:
                        emit_front(b)
                    if b >= 1:
                        emit_back(b - 1)

    nc.compile()
    return nc


# --------------------------------------------------------------------------
# host-side sharding / assembly (layout only, no FLOPs)
# --------------------------------------------------------------------------

def host_prepare(cfg: Cfg, h, src, jaccard, Wq, Wk, Wv):
    import ml_dtypes

    f32 = np.float32
    bf16 = ml_dtypes.bfloat16
    hT = np.ascontiguousarray(h.T.astype(f32, copy=False)).astype(bf16)

    def chunk2(w_t, width):  # [256, width] -> [128, 2, width]
        return np.ascontiguousarray(
            w_t.reshape(2, P, width).transpose(1, 0, 2)).astype(bf16)

    w_kv = chunk2(np.concatenate([Wk.T, Wv.T], axis=1).astype(f32), 384)
    w_q = chunk2(np.ascontiguousarray(Wq.T).astype(f32), P)

    # slot s = m*128 + p  <->  node n = c*NPC + s  (s < NPC valid)
    slots = np.arange(cfg.NPAD)
    s_grid = slots.reshape(cfg.M, P).T                     # [128, M]
    valid = s_grid < cfg.NPC

    in_maps = []
    for c in range(cfg.NC):
        base = c * cfg.NPC
        nodes = base + np.minimum(slots, cfg.NPC - 1)      # [NPAD]
        he = np.empty((8, 2, P, cfg.NPAD), dtype=bf16)
        jac = np.ones((P, 8, cfg.M), dtype=f32)
        for k in range(8):
            e = k * cfg.N + nodes
            he[k] = hT[:, src[e]].reshape(2, P, cfg.NPAD)
            jac[:, k, :] = np.where(valid, jaccard[e][s_grid], 1.0)
        hT_own = np.zeros((2, P, cfg.NPAD), dtype=bf16)
        span = min(cfg.NPAD, cfg.N - base)
        hT_own[:, :, :span] = hT[:, base:base + span].reshape(2, P, span)
        in_maps.append({
            "hT_own": hT_own,
            "he_t": he,
            "w_kv": w_kv,
            "w_q": w_q,
            "jac": np.ascontiguousarray(jac),
        })
    return in_maps


def assemble_output(cfg: Cfg, results):
    out = np.empty((cfg.N, NUM_HEADS, OUT_DIM), dtype=np.float32)
    for c, r in enumerate(results):
        shard = r["out"].reshape(P, cfg.M, 256).transpose(1, 0, 2).reshape(
            cfg.NPAD, 256)[:cfg.NPC]
        out[c * cfg.NPC:(c + 1) * cfg.NPC] = shard.reshape(
            cfg.NPC, NUM_HEADS, OUT_DIM)
    return out


# --------------------------------------------------------------------------
# numpy fallback (used only if inputs don't match the spec'd structure)
# --------------------------------------------------------------------------

def _numpy_reference(h, src, dst, jaccard, Wq, bq, Wk, bk, Wv, bv):
    N = h.shape[0]
    E = src.shape[0]
    h = h.astype(np.float32)
    Qh = (h @ Wq.T + bq).reshape(N, H2, OUT_DIM)
    Kh = (h @ Wk.T + bk).reshape(N, H2, OUT_DIM)
    Vh = (h @ Wv.T + bv).reshape(N, NUM_HEADS, OUT_DIM)
    score = np.sum(Kh[src] * Qh[dst], axis=-1, keepdims=True)
    score = np.exp(np.clip(score / np.sqrt(np.float32(OUT_DIM)), -5.0, 5.0))
    jac = np.broadcast_to(jaccard[:, None, None], (E, H2, 1))
    score_new = np.concatenate([score, jac], axis=1).astype(np.float32)
    contrib = (Vh[src] * score_new).astype(np.float32)
    wV = np.zeros((N, NUM_HEADS, OUT_DIM), dtype=np.float32)
    z = np.zeros((N, NUM_HEADS, 1), dtype=np.float32)
    np.add.at(wV, dst, contrib)
    np.add.at(z, dst, score_new)
    return wV / z


# --------------------------------------------------------------------------
# entry point
# --------------------------------------------------------------------------

_PROGRAM_CACHE = {}


def _get_program(cfg: Cfg):
    key = (cfg.N, cfg.E, cfg.NC)
    if key not in _PROGRAM_CACHE:
        _PROGRAM_CACHE[key] = build_program(cfg)
    return _PROGRAM_CACHE[key]


def _structure_ok(h, src, dst, jaccard, Wq, bq, Wk, bk, Wv, bv):
    if h.shape != (N_NODES, IN_DIM) or src.shape != (N_EDGES,):
        return False
    if Wq.shape != (H2 * OUT_DIM, IN_DIM) or Wv.shape != (NUM_HEADS * OUT_DIM, IN_DIM):
        return False
    if np.any(bq) or np.any(bk) or np.any(bv):
        return False
    if not np.array_equal(
            np.asarray(dst, dtype=np.int64),
            np.arange(N_EDGES, dtype=np.int64) % N_NODES):
        return False
    if src.min() < 0 or src.max() >= N_NODES:
        return False
    return True


def run_on_hw(inputs):
    from concourse.bass2jax import run_bass_via_pjrt

    cfg = FULL_CFG
    nc = _get_program(cfg)
    in_maps = host_prepare(
        cfg, inputs["h"], inputs["src"], inputs["jaccard"],
        inputs["Wq"], inputs["Wk"], inputs["Wv"])
    results = run_bass_via_pjrt(nc, in_maps, n_cores=cfg.NC)
    return assemble_output(cfg, results), results


def kernel(**inputs) -> np.ndarray:
    args = {k: np.asarray(v) for k, v in inputs.items()}
    if not _structure_ok(**args):
        return _numpy_reference(**args)
    out, _ = run_on_hw(args)
    return out


if __name__ == "__main__":
    print("building full program...")
    nc = _get_program(FULL_CFG)
    print("ok")


# revision 11
# speedup vs baseline: 1.0342x; 1.0251x over previous
"""Trainium2 Bass kernel for nn_MultiHeadAttentionLayer (GNN message passing).

Contract: kernel(**inputs) takes the FULL unsharded inputs (as produced by
setup_inputs()) and returns the FULL output [N, H, D] float32.

Strategy (8 NeuronCores, SPMD, no collectives, no gather):
  - dst == arange(E) % N (spec fill "arange"), so node n receives exactly
    E/N == 8 edges: e = k*N + n, k = 0..7.  Destination nodes are sharded
    across cores (N/8 nodes each); the host lays out the source-node
    features IN EDGE ORDER (pure indexing, no FLOPs), so the device streams
    them sequentially - the scatter/gather of the GNN disappears entirely.
  - Per (m-tile of 128 nodes, k): project the 128 gathered edge features
    through [Wk|Wv] (bf16, two 128-contraction matmuls) into PSUM.
    Scores: prod+reduce on DVE, exp on ACT.  Scores are pre-divided by
    z = sum_k score, so the aggregation needs no output divide.
  - V for the jaccard heads is scaled during PSUM eviction on the Scalar
    engine (activation Copy with per-partition scale = jaccard/z_jac);
    V for the QK heads is scaled on DVE.  The 8-edge accumulation is
    identity-matmuls into PSUM.  Blocks of 7 m-tiles are software-pipelined
    one deep so PE stays dense.
"""

import sys

import numpy as np

for _p in ("/opt/trn_rl_repo",):
    if _p not in sys.path:
        sys.path.insert(0, _p)

# --- problem constants (hardcoded per spec; kernel.py must be self-contained)
N_NODES = 50000
N_EDGES = 400000
IN_DIM = 256
OUT_DIM = 32
NUM_HEADS = 8
H2 = NUM_HEADS // 2
N_CORES = 8
P = 128

CLIP = 5.0 * np.sqrt(np.float32(32.0))  # clip on the raw dot product
SCALE = float(1.0 / np.sqrt(np.float32(32.0)))


class Cfg:
    def __init__(self, n_nodes=N_NODES, n_edges=N_EDGES, n_cores=N_CORES):
        assert n_edges == 8 * n_nodes
        self.N = n_nodes
        self.E = n_edges
        self.NC = n_cores
        assert n_nodes % n_cores == 0
        self.NPC = n_nodes // n_cores          # nodes per core
        self.M = -(-self.NPC // P)             # 128-node tiles per core
        self.NPAD = self.M * P                 # padded nodes per core
        self.BM = 7 if self.M % 7 == 0 else 1  # m-tiles per block
        self.NB = self.M // self.BM            # blocks


FULL_CFG = Cfg()


# --------------------------------------------------------------------------
# device program
# --------------------------------------------------------------------------

def build_program(cfg: Cfg, repeat: int = 1):
    import concourse.bacc as bacc
    import concourse.mybir as mybir
    import concourse.tile as tile
    from concourse.masks import make_identity

    f32 = mybir.dt.float32
    bf16 = mybir.dt.bfloat16
    M, BM, NB = cfg.M, cfg.BM, cfg.NB

    nc = bacc.Bacc(
        "TRN2",
        target_bir_lowering=False,
        debug=False,
        enable_asserts=False,
        num_devices=cfg.NC,
    )

    hT_own = nc.dram_tensor("hT_own", [2, P, cfg.NPAD], bf16, kind="ExternalInput")
    he_t = nc.dram_tensor("he_t", [8, 2, P, cfg.NPAD], bf16, kind="ExternalInput")
    w_kv = nc.dram_tensor("w_kv", [P, 2, 384], bf16, kind="ExternalInput")
    w_q = nc.dram_tensor("w_q", [P, 2, P], bf16, kind="ExternalInput")
    jac_t = nc.dram_tensor("jac", [P, 8, M], f32, kind="ExternalInput")
    out_t = nc.dram_tensor("out", [P, M * 256], f32, kind="ExternalOutput")

    mult = mybir.AluOpType.mult
    add = mybir.AluOpType.add
    amin = mybir.AluOpType.min
    amax = mybir.AluOpType.max
    EC = BM * P  # edge columns per (block, k)

    with tile.TileContext(nc) as tc:
        with tc.tile_pool(name="const", bufs=1) as const:
          for _rep in range(repeat):
            wkv_sb = const.tile([P, 2, 384], bf16, tag="wkv")
            wq_sb = const.tile([P, 2, P], bf16, tag="wq")
            jac_sb = const.tile([P, 8, M], f32, tag="jacsb")
            z47r = const.tile([P, M], f32, tag="z47r")
            jacw = const.tile([P, 8, M], f32, tag="jacw")
            qloc = const.tile([P, M, P], bf16, tag="qloc")
            ident = const.tile([P, P], bf16, tag="ident")

            nc.sync.dma_start(out=wkv_sb[:], in_=w_kv[:])
            nc.sync.dma_start(out=wq_sb[:], in_=w_q[:])
            nc.sync.dma_start(out=jac_sb[:], in_=jac_t[:])
            make_identity(nc, ident[:])

            # jacw = jac / sum_k jac  (pre-divided jaccard-head scores)
            nc.vector.tensor_reduce(
                out=z47r[:], in_=jac_sb[:].rearrange("p k m -> p m k"),
                axis=mybir.AxisListType.X, op=add)
            nc.vector.reciprocal(z47r[:], z47r[:])
            nc.vector.tensor_tensor(
                out=jacw[:], in0=jac_sb[:],
                in1=z47r[:].unsqueeze(1).to_broadcast([P, 8, M]), op=mult)

            # ---- phase A: Q projection of own node shard ------------------
            with (
                tc.tile_pool(name="pa", bufs=1) as pa,
                tc.tile_pool(name="pa_ps", bufs=2, space="PSUM") as pap,
            ):
                hto = pa.tile([P, 2, cfg.NPAD], bf16)
                nc.sync.dma_start(out=hto[:], in_=hT_own[:].rearrange("h f e -> f h e"))
                for t in range(M):
                    ps_q = pap.tile([P, P], f32)
                    nc.tensor.matmul(
                        out=ps_q[:], lhsT=hto[:, 0, t * P:(t + 1) * P],
                        rhs=wq_sb[:, 0, :], start=True, stop=False)
                    nc.tensor.matmul(
                        out=ps_q[:], lhsT=hto[:, 1, t * P:(t + 1) * P],
                        rhs=wq_sb[:, 1, :], start=False, stop=True)
                    nc.vector.tensor_copy(out=qloc[:, t, :], in_=ps_q[:])

            # ---- phase B: edge projection + attention, pipelined ----------
            with (
                tc.tile_pool(name="he", bufs=3) as hep,
                tc.tile_pool(name="kv_ps", bufs=3, space="PSUM") as kvp,
                tc.tile_pool(name="wv_ps", bufs=2, space="PSUM") as wvp,
                tc.tile_pool(name="pb", bufs=2) as pb,
                tc.tile_pool(name="po", bufs=2) as po,
            ):
                state = {}

                def emit_front(b):
                    """DMA + projection + prod + evictions for block b."""
                    gm0 = b * BM
                    prodb = pb.tile([P, BM, 8, P], bf16, tag="prodb")
                    vkq = pb.tile([P, BM, 8, P], bf16, tag="vkq")
                    sclb = pb.tile([P, 8, BM, 256], bf16, tag="sclb")
                    dots = pb.tile([P, BM, 8, 4], f32, tag="dots")
                    for k in range(8):
                        he = hep.tile([P, 2, EC], bf16, tag="he")
                        nc.sync.dma_start(
                            out=he[:],
                            in_=he_t[k, :, :, gm0 * P:gm0 * P + EC]
                                .rearrange("h f e -> f h e"))
                        for mp in range((BM + 1) // 2):
                            w = min(2, BM - 2 * mp)
                            kp = kvp.tile([P, 2, 512], f32, tag="kvps")
                            for i in range(w):
                                m = 2 * mp + i
                                nc.tensor.matmul(
                                    out=kp[:, i, 0:384],
                                    lhsT=he[:, 0, m * P:(m + 1) * P],
                                    rhs=wkv_sb[:, 0, :], start=True, stop=False)
                                nc.tensor.matmul(
                                    out=kp[:, i, 0:384],
                                    lhsT=he[:, 1, m * P:(m + 1) * P],
                                    rhs=wkv_sb[:, 1, :], start=False, stop=True)
                            # scores: K * Q  (bf16 products, reduced later)
                            nc.vector.tensor_tensor(
                                out=prodb[:, 2 * mp:2 * mp + w, k, :],
                                in0=kp[:, 0:w, 0:P],
                                in1=qloc[:, gm0 + 2 * mp:gm0 + 2 * mp + w, :],
                                op=mult)
                            # evict V (QK heads) to bf16
                            nc.scalar.copy(
                                out=vkq[:, 2 * mp:2 * mp + w, k, :],
                                in_=kp[:, 0:w, P:256])
                            # evict V (jac heads), scaled by jac/z on the fly
                            for i in range(w):
                                m = 2 * mp + i
                                nc.scalar.activation(
                                    out=sclb[:, k, m, P:256],
                                    in_=kp[:, i, 256:384],
                                    func=mybir.ActivationFunctionType.Copy,
                                    scale=jacw[:, k, gm0 + m:gm0 + m + 1])
                    state[b] = (prodb, vkq, sclb, dots)

                def emit_back(b):
                    """scores -> sew -> scl -> accumulate -> store, block b."""
                    gm0 = b * BM
                    prodb, vkq, sclb, dots = state.pop(b)
                    for m in range(BM):
                        nc.vector.tensor_reduce(
                            out=dots[:, m, :, :],
                            in_=prodb[:, m, :, :].rearrange(
                                "p k (h c) -> p k h c", c=32),
                            axis=mybir.AxisListType.X, op=add)
                    se = pb.tile([P, BM, 8, 4], f32, tag="se")
                    zq = pb.tile([P, BM, 4], f32, tag="zq")
                    sew = pb.tile([P, BM, 8, 4], bf16, tag="sew")
                    nc.vector.tensor_scalar(
                        out=dots[:], in0=dots[:], scalar1=float(CLIP),
                        scalar2=float(-CLIP), op0=amin, op1=amax)
                    nc.scalar.activation(
                        out=se[:], in_=dots[:],
                        func=mybir.ActivationFunctionType.Exp, scale=SCALE)
                    nc.vector.tensor_reduce(
                        out=zq[:], in_=se[:].rearrange("p m k h -> p m h k"),
                        axis=mybir.AxisListType.X, op=add)
                    nc.vector.reciprocal(zq[:], zq[:])
                    nc.vector.tensor_tensor(
                        out=sew[:], in0=se[:],
                        in1=zq[:].unsqueeze(2).to_broadcast([P, BM, 8, 4]),
                        op=mult)
                    for m in range(BM):
                        nc.vector.tensor_tensor(
                            out=sclb[:, :, m, 0:P].rearrange(
                                "p k (h c) -> p k h c", c=32),
                            in0=vkq[:, m, :, :].rearrange(
                                "p k (h c) -> p k h c", c=32),
                            in1=sew[:, m, :, :].unsqueeze(3)
                                .to_broadcast([P, 8, 4, 32]),
                            op=mult)
                    for mp in range((BM + 1) // 2):
                        w = min(2, BM - 2 * mp)
                        wv = wvp.tile([P, 2, 256], f32, tag="wv")
                        for k in range(8):
                            nc.tensor.matmul(
                                out=wv[:, 0:w, :].rearrange("p m f -> p (m f)"),
                                lhsT=ident[:],
                                rhs=sclb[:, k, 2 * mp:2 * mp + w, :].rearrange(
                                    "p m f -> p (m f)"),
                                start=(k == 0), stop=(k == 7))
                        ost = po.tile([P, 2, 256], f32, tag="ost")
                        nc.scalar.copy(out=ost[:, 0:w], in_=wv[:, 0:w])
                        nc.sync.dma_start(
                            out=out_t[:, (gm0 + 2 * mp) * 256:
                                       (gm0 + 2 * mp + w) * 256],
                            in_=ost[:, 0:w].rearrange("p m f -> p (m f)"))

                for b in range(NB + 1):
                    if b < NB:
                        emit_front(b)
                    if b >= 1:
                        emit_back(b - 1)

    nc.compile()
    return nc


# --------------------------------------------------------------------------
# host-side sharding / assembly (layout only, no FLOPs)
# --------------------------------------------------------------------------

def host_prepare(cfg: Cfg, h, src, jaccard, Wq, Wk, Wv):
    import ml_dtypes

    f32 = np.float32
    bf16 = ml_dtypes.bfloat16
    hT = np.ascontiguousarray(h.T.astype(f32, copy=False)).astype(bf16)

    def chunk2(w_t, width):  # [256, width] -> [128, 2, width]
        return np.ascontiguousarray(
            w_t.reshape(2, P, width).transpose(1, 0, 2)).astype(bf16)

    w_kv = chunk2(np.concatenate([Wk.T, Wv.T], axis=1).astype(f32), 384)
    w_q = chunk2(np.ascontiguousarray(Wq.T).astype(f32), P)

    # slot s = m*128 + p  <->  node n = c*NPC + s  (s < NPC valid)
    slots = np.arange(cfg.NPAD)
    s_grid = slots.reshape(cfg.M, P).T                     # [128, M]
    valid = s_grid < cfg.NPC

    in_maps = []
    for c in range(cfg.NC):
        base = c * cfg.NPC
        nodes = base + np.minimum(slots, cfg.NPC - 1)      # [NPAD]
        he = np.empty((8, 2, P, cfg.NPAD), dtype=bf16)
        jac = np.ones((P, 8, cfg.M), dtype=f32)
        for k in range(8):
            e = k * cfg.N + nodes
            he[k] = hT[:, src[e]].reshape(2, P, cfg.NPAD)
            jac[:, k, :] = np.where(valid, jaccard[e][s_grid], 1.0)
        hT_own = np.zeros((2, P, cfg.NPAD), dtype=bf16)
        span = min(cfg.NPAD, cfg.N - base)
        hT_own[:, :, :span] = hT[:, base:base + span].reshape(2, P, span)
        in_maps.append({
            "hT_own": hT_own,
            "he_t": he,
            "w_kv": w_kv,
            "w_q": w_q,
            "jac": np.ascontiguousarray(jac),
        })
    return in_maps


def assemble_output(cfg: Cfg, results):
    out = np.empty((cfg.N, NUM_HEADS, OUT_DIM), dtype=np.float32)
    for c, r in enumerate(results):
        shard = r["out"].reshape(P, cfg.M, 256).transpose(1, 0, 2).reshape(
            cfg.NPAD, 256)[:cfg.NPC]
        out[c * cfg.NPC:(c + 1) * cfg.NPC] = shard.reshape(
            cfg.NPC, NUM_HEADS, OUT_DIM)
    return out


# --------------------------------------------------------------------------
# numpy fallback (used only if inputs don't match the spec'd structure)
# --------------------------------------------------------------------------

def _numpy_reference(h, src, dst, jaccard, Wq, bq, Wk, bk, Wv, bv):
    N = h.shape[0]
    E = src.shape[0]
    h = h.astype(np.float32)
    Qh = (h @ Wq.T + bq).reshape(N, H2, OUT_DIM)
    Kh = (h @ Wk.T + bk).reshape(N, H2, OUT_DIM)
    Vh = (h @ Wv.T + bv).reshape(N, NUM_HEADS, OUT_DIM)
    score = np.sum(Kh[src] * Qh[dst], axis=-1, keepdims=True)
    score = np.exp(np.clip(score / np.sqrt(np.float32(OUT_DIM)), -5.0, 5.0))
    jac = np.broadcast_to(jaccard[:, None, None], (E, H2, 1))
    score_new = np.concatenate([score, jac], axis=1).astype(np.float32)
    contrib = (Vh[src] * score_new).astype(np.float32)
    wV = np.zeros((N, NUM_HEADS, OUT_DIM), dtype=np.float32)
    z = np.zeros((N, NUM_HEADS, 1), dtype=np.float32)
    np.add.at(wV, dst, contrib)
    np.add.at(z, dst, score_new)
    return wV / z


# --------------------------------------------------------------------------
# entry point
# --------------------------------------------------------------------------

_PROGRAM_CACHE = {}


def _get_program(cfg: Cfg):
    key = (cfg.N, cfg.E, cfg.NC)
    if key not in _PROGRAM_CACHE:
        _PROGRAM_CACHE[key] = build_program(cfg)
    return _PROGRAM_CACHE[key]


def _structure_ok(h, src, dst, jaccard, Wq, bq, Wk, bk, Wv, bv):
    if h.shape != (N_NODES, IN_DIM) or src.shape != (N_EDGES,):
        return False
    if Wq.shape != (H2 * OUT_DIM, IN_DIM) or Wv.shape != (NUM_HEADS * OUT_DIM, IN_DIM):
        return False
    if np.any(bq) or np.any(bk) or np.any(bv):
        return False
    if not np.array_equal(
            np.asarray(dst, dtype=np.int64),
            np.arange(N_EDGES, dtype=np.int64) % N_NODES):
        return False
    if src.min() < 0 or src.max() >= N_NODES:
        return False
    return True


def run_on_hw(inputs):
    from concourse.bass2jax import run_bass_via_pjrt

    cfg = FULL_CFG
    nc = _get_program(cfg)
    in_maps = host_prepare(
        cfg, inputs["h"], inputs["src"], inputs["jaccard"],
        inputs["Wq"], inputs["Wk"], inputs["Wv"])
    results = run_bass_via_pjrt(nc, in_maps, n_cores=cfg.NC)
    return assemble_output(cfg, results), results


def kernel(**inputs) -> np.ndarray:
    args = {k: np.asarray(v) for k, v in inputs.items()}
    if not _structure_ok(**args):
        return _numpy_reference(**args)
    out, _ = run_on_hw(args)
    return out


if __name__ == "__main__":
    print("building full program...")
    nc = _get_program(FULL_CFG)
    print("ok")


# revision 12
# speedup vs baseline: 1.4949x; 1.4454x over previous
"""Trainium2 Bass kernel for nn_MultiHeadAttentionLayer (GNN message passing).

Contract: kernel(**inputs) takes the FULL unsharded inputs (as produced by
setup_inputs()) and returns the FULL output [N, H, D] float32.

Strategy (8 NeuronCores, SPMD, no collectives, no gather):
  - dst == arange(E) % N (spec fill "arange"), so node n receives exactly
    E/N == 8 edges: e = k*N + n, k = 0..7.  Destination nodes are sharded
    across cores (N/8 nodes each); the host lays out the source-node
    features IN EDGE ORDER (pure indexing, no FLOPs), so the device streams
    them sequentially - the scatter/gather of the GNN disappears entirely.
  - Per (m-tile of 128 nodes, k): project the 128 gathered edge features
    through [Wk|Wv] (bf16, two 128-contraction matmuls) into PSUM.
    Scores: prod+reduce on DVE, exp on ACT.  Scores are pre-divided by
    z = sum_k score, so the aggregation needs no output divide.
  - V for the jaccard heads is scaled during PSUM eviction on the Scalar
    engine (activation Copy with per-partition scale = jaccard/z_jac);
    V for the QK heads is scaled on DVE.  The 8-edge accumulation is
    identity-matmuls into PSUM.  Blocks of 7 m-tiles are software-pipelined
    one deep so PE stays dense.
"""

import sys

import numpy as np

for _p in ("/opt/trn_rl_repo",):
    if _p not in sys.path:
        sys.path.insert(0, _p)

# --- problem constants (hardcoded per spec; kernel.py must be self-contained)
N_NODES = 50000
N_EDGES = 400000
IN_DIM = 256
OUT_DIM = 32
NUM_HEADS = 8
H2 = NUM_HEADS // 2
N_CORES = 8
P = 128

CLIP = 5.0 * np.sqrt(np.float32(32.0))  # clip on the raw dot product
SCALE = float(1.0 / np.sqrt(np.float32(32.0)))


class Cfg:
    def __init__(self, n_nodes=N_NODES, n_edges=N_EDGES, n_cores=N_CORES):
        assert n_edges == 8 * n_nodes
        self.N = n_nodes
        self.E = n_edges
        self.NC = n_cores
        assert n_nodes % n_cores == 0
        self.NPC = n_nodes // n_cores          # nodes per core
        self.M = -(-self.NPC // P)             # 128-node tiles per core
        self.NPAD = self.M * P                 # padded nodes per core
        self.BM = 7 if self.M % 7 == 0 else 1  # m-tiles per block
        self.NB = self.M // self.BM            # blocks


FULL_CFG = Cfg()


# --------------------------------------------------------------------------
# device program
# --------------------------------------------------------------------------

def build_program(cfg: Cfg, repeat: int = 1):
    import concourse.bacc as bacc
    import concourse.mybir as mybir
    import concourse.tile as tile
    from concourse.masks import make_identity

    f32 = mybir.dt.float32
    bf16 = mybir.dt.bfloat16
    M, BM, NB = cfg.M, cfg.BM, cfg.NB

    nc = bacc.Bacc(
        "TRN2",
        target_bir_lowering=False,
        debug=False,
        enable_asserts=False,
        num_devices=cfg.NC,
    )

    hT_own = nc.dram_tensor("hT_own", [2, P, cfg.NPAD], bf16, kind="ExternalInput")
    he_t = nc.dram_tensor("he_t", [8, 2, P, cfg.NPAD], bf16, kind="ExternalInput")
    w_kv = nc.dram_tensor("w_kv", [P, 2, 384], bf16, kind="ExternalInput")
    w_q = nc.dram_tensor("w_q", [P, 2, P], bf16, kind="ExternalInput")
    jac_t = nc.dram_tensor("jac", [P, 8, M], f32, kind="ExternalInput")
    out_t = nc.dram_tensor("out", [P, M * 256], f32, kind="ExternalOutput")

    mult = mybir.AluOpType.mult
    add = mybir.AluOpType.add
    amin = mybir.AluOpType.min
    amax = mybir.AluOpType.max
    EC = BM * P  # edge columns per (block, k)

    with tile.TileContext(nc) as tc:
        with tc.tile_pool(name="const", bufs=1) as const:
          for _rep in range(repeat):
            wkv_sb = const.tile([P, 2, 384], bf16, tag="wkv")
            wq_sb = const.tile([P, 2, P], bf16, tag="wq")
            jac_sb = const.tile([P, 8, M], f32, tag="jacsb")
            z47r = const.tile([P, M], f32, tag="z47r")
            jacw = const.tile([P, 8, M], f32, tag="jacw")
            qloc = const.tile([P, M, P], bf16, tag="qloc")
            ident = const.tile([P, P], bf16, tag="ident")

            nc.sync.dma_start(out=wkv_sb[:], in_=w_kv[:])
            nc.sync.dma_start(out=wq_sb[:], in_=w_q[:])
            nc.sync.dma_start(out=jac_sb[:], in_=jac_t[:])
            make_identity(nc, ident[:])

            # jacw = jac / sum_k jac  (pre-divided jaccard-head scores)
            nc.vector.tensor_reduce(
                out=z47r[:], in_=jac_sb[:].rearrange("p k m -> p m k"),
                axis=mybir.AxisListType.X, op=add)
            nc.vector.reciprocal(z47r[:], z47r[:])
            nc.vector.tensor_tensor(
                out=jacw[:], in0=jac_sb[:],
                in1=z47r[:].unsqueeze(1).to_broadcast([P, 8, M]), op=mult)

            # ---- phase A: Q projection of own node shard ------------------
            with (
                tc.tile_pool(name="pa", bufs=1) as pa,
                tc.tile_pool(name="pa_ps", bufs=2, space="PSUM") as pap,
            ):
                hto = pa.tile([P, 2, cfg.NPAD], bf16)
                nc.sync.dma_start(out=hto[:], in_=hT_own[:].rearrange("h f e -> f h e"))
                for t in range(M):
                    ps_q = pap.tile([P, P], f32)
                    nc.tensor.matmul(
                        out=ps_q[:], lhsT=hto[:, 0, t * P:(t + 1) * P],
                        rhs=wq_sb[:, 0, :], start=True, stop=False)
                    nc.tensor.matmul(
                        out=ps_q[:], lhsT=hto[:, 1, t * P:(t + 1) * P],
                        rhs=wq_sb[:, 1, :], start=False, stop=True)
                    nc.vector.tensor_copy(out=qloc[:, t, :], in_=ps_q[:])

            # ---- phase B: edge projection + attention, pipelined ----------
            with (
                tc.tile_pool(name="he", bufs=3) as hep,
                tc.tile_pool(name="kv_ps", bufs=3, space="PSUM") as kvp,
                tc.tile_pool(name="wv_ps", bufs=2, space="PSUM") as wvp,
                tc.tile_pool(name="pb", bufs=2) as pb,
                tc.tile_pool(name="po", bufs=2) as po,
            ):
                state = {}

                def emit_front(b):
                    """DMA + projection + prod + evictions for block b."""
                    gm0 = b * BM
                    prodb = pb.tile([P, BM, 8, P], bf16, tag="prodb")
                    vkq = pb.tile([P, BM, 8, P], bf16, tag="vkq")
                    sclb = pb.tile([P, 8, BM, 256], bf16, tag="sclb")
                    dots = pb.tile([P, BM, 8, 4], f32, tag="dots")
                    for k in range(8):
                        he = hep.tile([P, 2, EC], bf16, tag="he")
                        nc.sync.dma_start(
                            out=he[:],
                            in_=he_t[k, :, :, gm0 * P:gm0 * P + EC]
                                .rearrange("h f e -> f h e"))
                        for mp in range((BM + 1) // 2):
                            w = min(2, BM - 2 * mp)
                            kp = kvp.tile([P, 2, 512], f32, tag="kvps")
                            for i in range(w):
                                m = 2 * mp + i
                                nc.tensor.matmul(
                                    out=kp[:, i, 0:384],
                                    lhsT=he[:, 0, m * P:(m + 1) * P],
                                    rhs=wkv_sb[:, 0, :], start=True, stop=False)
                                nc.tensor.matmul(
                                    out=kp[:, i, 0:384],
                                    lhsT=he[:, 1, m * P:(m + 1) * P],
                                    rhs=wkv_sb[:, 1, :], start=False, stop=True)
                            # scores: K * Q  (bf16 products, reduced later)
                            nc.vector.tensor_tensor(
                                out=prodb[:, 2 * mp:2 * mp + w, k, :],
                                in0=kp[:, 0:w, 0:P],
                                in1=qloc[:, gm0 + 2 * mp:gm0 + 2 * mp + w, :],
                                op=mult)
                            # evict V (QK heads) to bf16
                            nc.scalar.copy(
                                out=vkq[:, 2 * mp:2 * mp + w, k, :],
                                in_=kp[:, 0:w, P:256])
                            # evict V (jac heads), scaled by jac/z on the fly;
                            # k 6-7 on DVE to shorten the Scalar chain
                            for i in range(w):
                                m = 2 * mp + i
                                if k >= 6:
                                    nc.vector.tensor_tensor(
                                        out=sclb[:, k, m, P:256],
                                        in0=kp[:, i, 256:384],
                                        in1=jacw[:, k, gm0 + m:gm0 + m + 1]
                                            .to_broadcast([P, P]),
                                        op=mult)
                                else:
                                    nc.scalar.activation(
                                        out=sclb[:, k, m, P:256],
                                        in_=kp[:, i, 256:384],
                                        func=mybir.ActivationFunctionType.Copy,
                                        scale=jacw[:, k, gm0 + m:gm0 + m + 1])
                    state[b] = (prodb, vkq, sclb, dots)

                def emit_back(b):
                    """scores -> sew -> scl -> accumulate -> store, block b."""
                    gm0 = b * BM
                    prodb, vkq, sclb, dots = state.pop(b)
                    for m in range(BM):
                        nc.vector.tensor_reduce(
                            out=dots[:, m, :, :],
                            in_=prodb[:, m, :, :].rearrange(
                                "p k (h c) -> p k h c", c=32),
                            axis=mybir.AxisListType.X, op=add)
                    se = pb.tile([P, BM, 8, 4], f32, tag="se")
                    zq = pb.tile([P, BM, 4], f32, tag="zq")
                    sew = pb.tile([P, BM, 8, 4], bf16, tag="sew")
                    nc.vector.tensor_scalar(
                        out=dots[:], in0=dots[:], scalar1=float(CLIP),
                        scalar2=float(-CLIP), op0=amin, op1=amax)
                    nc.scalar.activation(
                        out=se[:], in_=dots[:],
                        func=mybir.ActivationFunctionType.Exp, scale=SCALE)
                    nc.vector.tensor_reduce(
                        out=zq[:], in_=se[:].rearrange("p m k h -> p m h k"),
                        axis=mybir.AxisListType.X, op=add)
                    nc.vector.reciprocal(zq[:], zq[:])
                    nc.vector.tensor_tensor(
                        out=sew[:], in0=se[:],
                        in1=zq[:].unsqueeze(2).to_broadcast([P, BM, 8, 4]),
                        op=mult)
                    for m in range(BM):
                        nc.vector.tensor_tensor(
                            out=sclb[:, :, m, 0:P].rearrange(
                                "p k (h c) -> p k h c", c=32),
                            in0=vkq[:, m, :, :].rearrange(
                                "p k (h c) -> p k h c", c=32),
                            in1=sew[:, m, :, :].unsqueeze(3)
                                .to_broadcast([P, 8, 4, 32]),
                            op=mult)
                    for mp in range((BM + 1) // 2):
                        w = min(2, BM - 2 * mp)
                        wv = wvp.tile([P, 2, 256], f32, tag="wv")
                        for k in range(8):
                            nc.tensor.matmul(
                                out=wv[:, 0:w, :].rearrange("p m f -> p (m f)"),
                                lhsT=ident[:],
                                rhs=sclb[:, k, 2 * mp:2 * mp + w, :].rearrange(
                                    "p m f -> p (m f)"),
                                start=(k == 0), stop=(k == 7))
                        ost = po.tile([P, 2, 256], f32, tag="ost")
                        nc.scalar.copy(out=ost[:, 0:w], in_=wv[:, 0:w])
                        nc.sync.dma_start(
                            out=out_t[:, (gm0 + 2 * mp) * 256:
                                       (gm0 + 2 * mp + w) * 256],
                            in_=ost[:, 0:w].rearrange("p m f -> p (m f)"))

                for b in range(NB + 1):
                    if b < NB:
                        emit_front(b)
                    if b >= 1:
                        emit_back(b - 1)

    nc.compile()
    return nc


# --------------------------------------------------------------------------
# host-side sharding / assembly (layout only, no FLOPs)
# --------------------------------------------------------------------------

def host_prepare(cfg: Cfg, h, src, jaccard, Wq, Wk, Wv):
    import ml_dtypes

    f32 = np.float32
    bf16 = ml_dtypes.bfloat16
    hT = np.ascontiguousarray(h.T.astype(f32, copy=False)).astype(bf16)

    def chunk2(w_t, width):  # [256, width] -> [128, 2, width]
        return np.ascontiguousarray(
            w_t.reshape(2, P, width).transpose(1, 0, 2)).astype(bf16)

    w_kv = chunk2(np.concatenate([Wk.T, Wv.T], axis=1).astype(f32), 384)
    w_q = chunk2(np.ascontiguousarray(Wq.T).astype(f32), P)

    # slot s = m*128 + p  <->  node n = c*NPC + s  (s < NPC valid)
    slots = np.arange(cfg.NPAD)
    s_grid = slots.reshape(cfg.M, P).T                     # [128, M]
    valid = s_grid < cfg.NPC

    in_maps = []
    for c in range(cfg.NC):
        base = c * cfg.NPC
        nodes = base + np.minimum(slots, cfg.NPC - 1)      # [NPAD]
        he = np.empty((8, 2, P, cfg.NPAD), dtype=bf16)
        jac = np.ones((P, 8, cfg.M), dtype=f32)
        for k in range(8):
            e = k * cfg.N + nodes
            he[k] = hT[:, src[e]].reshape(2, P, cfg.NPAD)
            jac[:, k, :] = np.where(valid, jaccard[e][s_grid], 1.0)
        hT_own = np.zeros((2, P, cfg.NPAD), dtype=bf16)
        span = min(cfg.NPAD, cfg.N - base)
        hT_own[:, :, :span] = hT[:, base:base + span].reshape(2, P, span)
        in_maps.append({
            "hT_own": hT_own,
            "he_t": he,
            "w_kv": w_kv,
            "w_q": w_q,
            "jac": np.ascontiguousarray(jac),
        })
    return in_maps


def assemble_output(cfg: Cfg, results):
    out = np.empty((cfg.N, NUM_HEADS, OUT_DIM), dtype=np.float32)
    for c, r in enumerate(results):
        shard = r["out"].reshape(P, cfg.M, 256).transpose(1, 0, 2).reshape(
            cfg.NPAD, 256)[:cfg.NPC]
        out[c * cfg.NPC:(c + 1) * cfg.NPC] = shard.reshape(
            cfg.NPC, NUM_HEADS, OUT_DIM)
    return out


# --------------------------------------------------------------------------
# numpy fallback (used only if inputs don't match the spec'd structure)
# --------------------------------------------------------------------------

def _numpy_reference(h, src, dst, jaccard, Wq, bq, Wk, bk, Wv, bv):
    N = h.shape[0]
    E = src.shape[0]
    h = h.astype(np.float32)
    Qh = (h @ Wq.T + bq).reshape(N, H2, OUT_DIM)
    Kh = (h @ Wk.T + bk).reshape(N, H2, OUT_DIM)
    Vh = (h @ Wv.T + bv).reshape(N, NUM_HEADS, OUT_DIM)
    score = np.sum(Kh[src] * Qh[dst], axis=-1, keepdims=True)
    score = np.exp(np.clip(score / np.sqrt(np.float32(OUT_DIM)), -5.0, 5.0))
    jac = np.broadcast_to(jaccard[:, None, None], (E, H2, 1))
    score_new = np.concatenate([score, jac], axis=1).astype(np.float32)
    contrib = (Vh[src] * score_new).astype(np.float32)
    wV = np.zeros((N, NUM_HEADS, OUT_DIM), dtype=np.float32)
    z = np.zeros((N, NUM_HEADS, 1), dtype=np.float32)
    np.add.at(wV, dst, contrib)
    np.add.at(z, dst, score_new)
    return wV / z


# --------------------------------------------------------------------------
# entry point
# --------------------------------------------------------------------------

_PROGRAM_CACHE = {}


def _get_program(cfg: Cfg):
    key = (cfg.N, cfg.E, cfg.NC)
    if key not in _PROGRAM_CACHE:
        _PROGRAM_CACHE[key] = build_program(cfg)
    return _PROGRAM_CACHE[key]


def _structure_ok(h, src, dst, jaccard, Wq, bq, Wk, bk, Wv, bv):
    if h.shape != (N_NODES, IN_DIM) or src.shape != (N_EDGES,):
        return False
    if Wq.shape != (H2 * OUT_DIM, IN_DIM) or Wv.shape != (NUM_HEADS * OUT_DIM, IN_DIM):
        return False
    if np.any(bq) or np.any(bk) or np.any(bv):
        return False
    if not np.array_equal(
            np.asarray(dst, dtype=np.int64),
            np.arange(N_EDGES, dtype=np.int64) % N_NODES):
        return False
    if src.min() < 0 or src.max() >= N_NODES:
        return False
    return True


def run_on_hw(inputs):
    from concourse.bass2jax import run_bass_via_pjrt

    cfg = FULL_CFG
    nc = _get_program(cfg)
    in_maps = host_prepare(
        cfg, inputs["h"], inputs["src"], inputs["jaccard"],
        inputs["Wq"], inputs["Wk"], inputs["Wv"])
    results = run_bass_via_pjrt(nc, in_maps, n_cores=cfg.NC)
    return assemble_output(cfg, results), results


def kernel(**inputs) -> np.ndarray:
    args = {k: np.asarray(v) for k, v in inputs.items()}
    if not _structure_ok(**args):
        return _numpy_reference(**args)
    out, _ = run_on_hw(args)
    return out


if __name__ == "__main__":
    print("building full program...")
    nc = _get_program(FULL_CFG)
    print("ok")
